# revision 16
# baseline (speedup 1.0000x reference)
"""Trainium2 kernel for nn_CrossLayerLateral.

out[b,s,i] = x_current[b,s,i] + alpha * sum_j x_prev[b,s,j] * W[i,j]
with W built from COO (duplicates summed).

Strategy (data-parallel over tokens, 8 cores):
  - Host: build WT[j,i] = W[i,j] via bincount scatter-add (cheap, O(nnz));
    cast WT and x_prev to a low-precision matmul dtype. The quantization
    error lands only on the alpha-scaled lateral term (~1.4e-4 of |out|),
    so even fp8 keeps the final output within ~1e-5 relative.
  - Flatten (B,S) -> 8192 tokens, 1024 tokens per core. Each core gets:
      xc  [1024, 2048] f32       x_current slice (token-major)
      wx  [16, 128, 3072] lp     per j-chunk: [scaled W^T row | x_prev^T],
                                 j on partitions, W^T replicated
      out [1024, 2048] f32
  - Device: psum = xpt.T @ wt accumulated over j-chunks in PSUM (fp32),
    DVE fused (psum * scale + xc), DMA out per 512-column block.
  - Token tiles are processed in pairs (8 psum groups = all 8 banks), so
    during the weight-streaming phase the PE gets 8 matmuls of work per
    arriving j-chunk.
  - FP8 path (default): e4m3 operands with perf_mode=DoubleRow (virtual
    128x256 array) halves the matmul stream; W^T is pre-scaled by 2**10
    into the e4m3 normal range and alpha/2**10 is folded into the DVE op.
"""
import numpy as np
import ml_dtypes

import concourse.bass as bass
import concourse.tile as tile
from concourse import bacc, mybir
from concourse.bass_utils import run_bass_kernel_spmd

H = 2048          # hidden
B, S = 4, 2048
TOK = B * S       # 8192 tokens
NCORES = 8
TPC = TOK // NCORES   # 1024 tokens per core
P = 128
JC = H // P       # 16 j-chunks (contraction)
ST = TPC // P     # 8 token tiles per core
NB = 512          # matmul free dim / psum bank
IB = H // NB      # 4 output column blocks
NBLK = ST // 2    # token-tile pairs
CW = H + TPC      # combined per-chunk row: [wt row | xpt row]
WARMUP_MM = 20    # HAM warmup matmuls while weights stream in

FP8 = True        # False -> bf16 matmul path
W_SCALE = 1024.0  # fp8: pre-scale for W^T into e4m3 normal range

_NC_CACHE = {}


def build_nc(fp8=FP8, alpha=1.0):
    lp_dt = mybir.dt.float8e4 if fp8 else mybir.dt.bfloat16
    perf_mode = mybir.MatmulPerfMode.DoubleRow if fp8 else None
    kstep = 2 if fp8 else 1   # j-chunks consumed per matmul

    nc = bacc.Bacc("TRN2", target_bir_lowering=False, debug=False,
                   enable_asserts=False, num_devices=NCORES)
    xc = nc.dram_tensor("xc", [TPC, H], mybir.dt.float32,
                        kind="ExternalInput").ap()
    wx = nc.dram_tensor("wx", [JC, P, CW], lp_dt,
                        kind="ExternalInput").ap()
    out = nc.dram_tensor("out", [TPC, H], mybir.dt.float32,
                         kind="ExternalOutput").ap()

    with tile.TileContext(nc) as tc:
        with (
            tc.tile_pool(name="weights", bufs=1) as wpool,
            tc.tile_pool(name="io", bufs=3) as io,
            tc.tile_pool(name="psum", bufs=2, space="PSUM") as psum,
        ):
            # PE warmup: garbage matmuls with no DMA deps keep the HAM
            # activity window busy while the first weight chunks stream in.
            wu = wpool.tile([P, P], mybir.dt.bfloat16, name="wu", tag="wu")
            nc.vector.memset(wu[:], 0.0)
            wu_ps = psum.tile([P, P], mybir.dt.float32, name="wups",
                              tag="ps0")
            for _ in range(WARMUP_MM):
                nc.tensor.matmul(wu_ps[:], lhsT=wu[:], rhs=wu[:],
                                 start=True, stop=True)

            # Resident weights+activations in one 3D tile (DoubleRow needs
            # [P, 2, n] APs spanning adjacent j-chunks), DMA'd per j-chunk
            # so matmuls start as chunks land. Alternate the two HWDGE
            # issue engines (sync / scalar). x_current tiles for the first
            # block are slotted into the weight stream so the first adds
            # aren't gated on the tail of the weight transfer.
            wx_sb = wpool.tile([P, JC, CW], lp_dt, name="wxs", tag="wxs")
            xc_pend = {}

            def load_xc(st, eng):
                t = io.tile([P, H], mybir.dt.float32, name=f"xc{st}",
                            tag=f"xc{st % 2}")
                eng.dma_start(t[:], xc[bass.ts(st, P), :])
                return t

            for jo in range(JC):
                eng = nc.sync if jo % 2 == 0 else nc.scalar
                eng.dma_start(wx_sb[:, jo, :], wx[jo])
            xc_pend[0] = load_xc(0, nc.sync)
            xc_pend[1] = load_xc(1, nc.scalar)

            for blk in range(NBLK):
                sts = (2 * blk, 2 * blk + 1)
                xc_ts = []
                for k, st in enumerate(sts):
                    if st in xc_pend:
                        xc_ts.append(xc_pend.pop(st))
                    else:
                        xc_ts.append(load_xc(st, nc.sync if k == 0
                                             else nc.scalar))
                out_ts = [io.tile([P, H], mybir.dt.float32, name=f"o{st}",
                                  tag=f"out{k}") for k, st in enumerate(sts)]
                # 8 psum groups per block over 4 tags x 2 slots = 8 banks
                ps = [psum.tile([P, NB], mybir.dt.float32,
                                name=f"ps{blk}_{g}", tag=f"ps{g % 4}")
                      for g in range(2 * IB)]

                def mm(g, jp):
                    k, ib = divmod(g, IB)
                    jsl = slice(jp * kstep, (jp + 1) * kstep)
                    lhsT = wx_sb[:, jsl, H + sts[k] * P:H + (sts[k] + 1) * P]
                    rhs = wx_sb[:, jsl, ib * NB:(ib + 1) * NB]
                    if not fp8:
                        lhsT = lhsT[:, 0]
                        rhs = rhs[:, 0]
                    nc.tensor.matmul(
                        ps[g][:], lhsT=lhsT, rhs=rhs,
                        start=(jp == 0), stop=(jp == JC // kstep - 1),
                        perf_mode=perf_mode,
                    )

                if blk == 0:
                    # Weight-streaming phase: consume j-chunks in arrival
                    # order, 8 matmuls of work per chunk.
                    for jp in range(JC // kstep):
                        for g in range(2 * IB):
                            mm(g, jp)
                else:
                    # Resident phase: finish groups one at a time so the
                    # adds/stores stagger across the whole block.
                    for g in range(2 * IB):
                        for jp in range(JC // kstep):
                            mm(g, jp)

                # Block A only: two-step epilogue — the scaled psum->sbuf
                # copy frees the bank without waiting for x_current (which
                # trails the weight stream in the DMA queue), so block B's
                # matmuls start immediately and HAM stays warm. Later
                # blocks use the fused (psum*scale + xc) op: their xc
                # tiles land well before their matmuls finish.
                if blk == 0:
                    for g in range(2 * IB):
                        k, ib = divmod(g, IB)
                        isl = bass.ts(ib, NB)
                        if fp8:
                            nc.vector.tensor_scalar_mul(
                                out_ts[k][:, isl], ps[g][:],
                                float(alpha) / W_SCALE)
                        else:
                            nc.vector.tensor_copy(out_ts[k][:, isl],
                                                  ps[g][:])
                    for g in range(2 * IB):
                        k, ib = divmod(g, IB)
                        isl = bass.ts(ib, NB)
                        nc.vector.tensor_add(out_ts[k][:, isl],
                                             out_ts[k][:, isl],
                                             xc_ts[k][:, isl])
                        eng = nc.sync if g % 2 == 0 else nc.scalar
                        eng.dma_start(out[bass.ts(sts[k], P), isl],
                                      out_ts[k][:, isl])
                else:
                    for g in range(2 * IB):
                        k, ib = divmod(g, IB)
                        isl = bass.ts(ib, NB)
                        if fp8:
                            nc.vector.scalar_tensor_tensor(
                                out_ts[k][:, isl], ps[g][:],
                                float(alpha) / W_SCALE, xc_ts[k][:, isl],
                                mybir.AluOpType.mult, mybir.AluOpType.add)
                        else:
                            nc.vector.tensor_add(out_ts[k][:, isl],
                                                 ps[g][:], xc_ts[k][:, isl])
                        eng = nc.sync if g % 2 == 0 else nc.scalar
                        eng.dma_start(out[bass.ts(sts[k], P), isl],
                                      out_ts[k][:, isl])

    nc.compile()
    return nc


def _get_nc(fp8=FP8, alpha=1.0):
    # alpha is baked into the fp8 epilogue immediate, so key the cache on it
    key = (fp8, float(alpha) if fp8 else None)
    if key not in _NC_CACHE:
        _NC_CACHE[key] = build_nc(fp8, alpha=alpha)
    return _NC_CACHE[key]


def _prep_inputs(x_current, x_prev, alpha, connection_values,
                 connection_indices, fp8=FP8):
    lp_np = mybir.dt.np(mybir.dt.float8e4) if fp8 else ml_dtypes.bfloat16
    # WT[j, i] = W[i, j];  W[r, c] += v  =>  WT[c, r] += v
    r = connection_indices[0].astype(np.int64)
    c = connection_indices[1].astype(np.int64)
    wt_flat = np.bincount(c * H + r, weights=connection_values.astype(np.float64),
                          minlength=H * H).astype(np.float32)
    if fp8:
        wt_lp = (np.float32(W_SCALE) * wt_flat).astype(lp_np).reshape(H, H)
    else:
        # fold alpha into the bf16 weights; DVE does a plain add
        wt_lp = (np.float32(alpha) * wt_flat).astype(lp_np).reshape(H, H)

    xp = np.asarray(x_prev, dtype=np.float32).reshape(TOK, H)
    xc = np.ascontiguousarray(np.asarray(x_current,
                                         dtype=np.float32).reshape(TOK, H))
    in_maps = []
    for core in range(NCORES):
        sl = slice(core * TPC, (core + 1) * TPC)
        wx_core = np.empty((H, CW), dtype=lp_np)
        wx_core[:, :H] = wt_lp
        wx_core[:, H:] = xp[sl].T.astype(lp_np)
        in_maps.append({
            "xc": np.ascontiguousarray(xc[sl]),
            "wx": wx_core.reshape(JC, P, CW),
        })
    return in_maps


def kernel(x_current, x_prev, alpha, connection_values, connection_indices):
    fp8 = FP8
    nc = _get_nc(fp8, alpha=alpha)
    in_maps = _prep_inputs(x_current, x_prev, alpha, connection_values,
                           connection_indices, fp8)
    res = run_bass_kernel_spmd(nc, in_maps, list(range(NCORES)))
    out = np.concatenate([res.results[i]["out"] for i in range(NCORES)], axis=0)
    return out.reshape(B, S, H).astype(np.float32, copy=False)


# revision 17
# speedup vs baseline: 1.0996x; 1.0996x over previous
"""Trainium2 kernel for nn_CrossLayerLateral.

out[b,s,i] = x_current[b,s,i] + alpha * sum_j x_prev[b,s,j] * W[i,j]
with W built from COO (duplicates summed).

Strategy (data-parallel over tokens, 8 cores):
  - Host: build WT[j,i] = W[i,j] via bincount scatter-add (cheap, O(nnz));
    cast WT and x_prev to a low-precision matmul dtype. The quantization
    error lands only on the alpha-scaled lateral term (~1.4e-4 of |out|),
    so even fp8 keeps the final output within ~1e-5 relative.
  - Flatten (B,S) -> 8192 tokens, 1024 tokens per core. Each core gets:
      xc  [1024, 2048] f32       x_current slice (token-major)
      wx  [16, 128, 3072] lp     per j-chunk: [scaled W^T row | x_prev^T],
                                 j on partitions, W^T replicated
      out [1024, 2048] f32
  - Device: psum = xpt.T @ wt accumulated over j-chunks in PSUM (fp32),
    DVE fused (psum * scale + xc), DMA out per 512-column block.
  - Token tiles are processed in pairs (8 psum groups = all 8 banks), so
    during the weight-streaming phase the PE gets 8 matmuls of work per
    arriving j-chunk.
  - FP8 path (default): e4m3 operands with perf_mode=DoubleRow (virtual
    128x256 array) halves the matmul stream; W^T is pre-scaled by 2**10
    into the e4m3 normal range and alpha/2**10 is folded into the DVE op.
"""
import numpy as np
import ml_dtypes

import concourse.bass as bass
import concourse.tile as tile
from concourse import bacc, mybir
from concourse.bass_utils import run_bass_kernel_spmd

H = 2048          # hidden
B, S = 4, 2048
TOK = B * S       # 8192 tokens
NCORES = 8
TPC = TOK // NCORES   # 1024 tokens per core
P = 128
JC = H // P       # 16 j-chunks (contraction)
ST = TPC // P     # 8 token tiles per core
NB = 512          # matmul free dim / psum bank
IB = H // NB      # 4 output column blocks
NBLK = ST // 2    # token-tile pairs
CW = H + TPC      # combined per-chunk row: [wt row | xpt row]
WARMUP_MM = 12    # HAM warmup matmuls while weights stream in

FP8 = True        # False -> bf16 matmul path
W_SCALE = 1024.0  # fp8: pre-scale for W^T into e4m3 normal range

_NC_CACHE = {}


def build_nc(fp8=FP8, alpha=1.0):
    lp_dt = mybir.dt.float8e4 if fp8 else mybir.dt.bfloat16
    perf_mode = mybir.MatmulPerfMode.DoubleRow if fp8 else None
    kstep = 2 if fp8 else 1   # j-chunks consumed per matmul

    nc = bacc.Bacc("TRN2", target_bir_lowering=False, debug=False,
                   enable_asserts=False, num_devices=NCORES)
    xc = nc.dram_tensor("xc", [TPC, H], mybir.dt.float32,
                        kind="ExternalInput").ap()
    wx = nc.dram_tensor("wx", [JC, P, CW], lp_dt,
                        kind="ExternalInput").ap()
    out = nc.dram_tensor("out", [TPC, H], mybir.dt.float32,
                         kind="ExternalOutput").ap()

    with tile.TileContext(nc) as tc:
        with (
            tc.tile_pool(name="weights", bufs=1) as wpool,
            tc.tile_pool(name="io", bufs=3) as io,
            tc.tile_pool(name="psum", bufs=2, space="PSUM") as psum,
        ):
            # PE warmup: garbage matmuls with no DMA deps keep the HAM
            # activity window busy while the first weight chunks stream in.
            wu = wpool.tile([P, P], mybir.dt.bfloat16, name="wu", tag="wu")
            nc.vector.memset(wu[:], 0.0)
            wu_ps = psum.tile([P, P], mybir.dt.float32, name="wups",
                              tag="ps0")
            for _ in range(WARMUP_MM):
                nc.tensor.matmul(wu_ps[:], lhsT=wu[:], rhs=wu[:],
                                 start=True, stop=True)

            # Resident weights+activations in one 3D tile (DoubleRow needs
            # [P, 2, n] APs spanning adjacent j-chunks), DMA'd per j-chunk
            # so matmuls start as chunks land. Alternate the two HWDGE
            # issue engines (sync / scalar). x_current tiles for the first
            # block are slotted into the weight stream so the first adds
            # aren't gated on the tail of the weight transfer.
            wx_sb = wpool.tile([P, JC, CW], lp_dt, name="wxs", tag="wxs")
            xc_pend = {}

            def load_xc(st, eng):
                t = io.tile([P, H], mybir.dt.float32, name=f"xc{st}",
                            tag=f"xc{st % 2}")
                eng.dma_start(t[:], xc[bass.ts(st, P), :])
                return t

            for jo in range(JC):
                eng = nc.sync if jo % 2 == 0 else nc.scalar
                eng.dma_start(wx_sb[:, jo, :], wx[jo])
            xc_pend[0] = load_xc(0, nc.sync)
            xc_pend[1] = load_xc(1, nc.scalar)

            for blk in range(NBLK):
                sts = (2 * blk, 2 * blk + 1)
                xc_ts = []
                for k, st in enumerate(sts):
                    if st in xc_pend:
                        xc_ts.append(xc_pend.pop(st))
                    else:
                        xc_ts.append(load_xc(st, nc.sync if k == 0
                                             else nc.scalar))
                out_ts = [io.tile([P, H], mybir.dt.float32, name=f"o{st}",
                                  tag=f"out{k}") for k, st in enumerate(sts)]
                # 8 psum groups per block over 4 tags x 2 slots = 8 banks
                ps = [psum.tile([P, NB], mybir.dt.float32,
                                name=f"ps{blk}_{g}", tag=f"ps{g % 4}")
                      for g in range(2 * IB)]

                def mm(g, jp):
                    k, ib = divmod(g, IB)
                    jsl = slice(jp * kstep, (jp + 1) * kstep)
                    lhsT = wx_sb[:, jsl, H + sts[k] * P:H + (sts[k] + 1) * P]
                    rhs = wx_sb[:, jsl, ib * NB:(ib + 1) * NB]
                    if not fp8:
                        lhsT = lhsT[:, 0]
                        rhs = rhs[:, 0]
                    nc.tensor.matmul(
                        ps[g][:], lhsT=lhsT, rhs=rhs,
                        start=(jp == 0), stop=(jp == JC // kstep - 1),
                        perf_mode=perf_mode,
                    )

                if blk == 0:
                    # Weight-streaming phase: consume j-chunks in arrival
                    # order, 8 matmuls of work per chunk.
                    for jp in range(JC // kstep):
                        for g in range(2 * IB):
                            mm(g, jp)
                else:
                    # Resident phase: finish groups one at a time so the
                    # adds/stores stagger across the whole block.
                    for g in range(2 * IB):
                        for jp in range(JC // kstep):
                            mm(g, jp)

                # Block A only: two-step epilogue — the scaled psum->sbuf
                # copy frees the bank without waiting for x_current (which
                # trails the weight stream in the DMA queue), so block B's
                # matmuls start immediately and HAM stays warm. Later
                # blocks use the fused (psum*scale + xc) op: their xc
                # tiles land well before their matmuls finish.
                if blk == 0:
                    for g in range(2 * IB):
                        k, ib = divmod(g, IB)
                        isl = bass.ts(ib, NB)
                        if fp8:
                            nc.vector.tensor_scalar_mul(
                                out_ts[k][:, isl], ps[g][:],
                                float(alpha) / W_SCALE)
                        else:
                            nc.vector.tensor_copy(out_ts[k][:, isl],
                                                  ps[g][:])
                    for g in range(2 * IB):
                        k, ib = divmod(g, IB)
                        isl = bass.ts(ib, NB)
                        nc.vector.tensor_add(out_ts[k][:, isl],
                                             out_ts[k][:, isl],
                                             xc_ts[k][:, isl])
                        eng = nc.sync if g % 2 == 0 else nc.scalar
                        eng.dma_start(out[bass.ts(sts[k], P), isl],
                                      out_ts[k][:, isl])
                else:
                    for g in range(2 * IB):
                        k, ib = divmod(g, IB)
                        isl = bass.ts(ib, NB)
                        if fp8:
                            nc.vector.scalar_tensor_tensor(
                                out_ts[k][:, isl], ps[g][:],
                                float(alpha) / W_SCALE, xc_ts[k][:, isl],
                                mybir.AluOpType.mult, mybir.AluOpType.add)
                        else:
                            nc.vector.tensor_add(out_ts[k][:, isl],
                                                 ps[g][:], xc_ts[k][:, isl])
                        eng = nc.sync if g % 2 == 0 else nc.scalar
                        eng.dma_start(out[bass.ts(sts[k], P), isl],
                                      out_ts[k][:, isl])

    nc.compile()
    return nc


def _get_nc(fp8=FP8, alpha=1.0):
    # alpha is baked into the fp8 epilogue immediate, so key the cache on it
    key = (fp8, float(alpha) if fp8 else None)
    if key not in _NC_CACHE:
        _NC_CACHE[key] = build_nc(fp8, alpha=alpha)
    return _NC_CACHE[key]


def _prep_inputs(x_current, x_prev, alpha, connection_values,
                 connection_indices, fp8=FP8):
    lp_np = mybir.dt.np(mybir.dt.float8e4) if fp8 else ml_dtypes.bfloat16
    # WT[j, i] = W[i, j];  W[r, c] += v  =>  WT[c, r] += v
    r = connection_indices[0].astype(np.int64)
    c = connection_indices[1].astype(np.int64)
    wt_flat = np.bincount(c * H + r, weights=connection_values.astype(np.float64),
                          minlength=H * H).astype(np.float32)
    if fp8:
        wt_lp = (np.float32(W_SCALE) * wt_flat).astype(lp_np).reshape(H, H)
    else:
        # fold alpha into the bf16 weights; DVE does a plain add
        wt_lp = (np.float32(alpha) * wt_flat).astype(lp_np).reshape(H, H)

    xp = np.asarray(x_prev, dtype=np.float32).reshape(TOK, H)
    xc = np.ascontiguousarray(np.asarray(x_current,
                                         dtype=np.float32).reshape(TOK, H))
    in_maps = []
    for core in range(NCORES):
        sl = slice(core * TPC, (core + 1) * TPC)
        wx_core = np.empty((H, CW), dtype=lp_np)
        wx_core[:, :H] = wt_lp
        wx_core[:, H:] = xp[sl].T.astype(lp_np)
        in_maps.append({
            "xc": np.ascontiguousarray(xc[sl]),
            "wx": wx_core.reshape(JC, P, CW),
        })
    return in_maps


def kernel(x_current, x_prev, alpha, connection_values, connection_indices):
    fp8 = FP8
    nc = _get_nc(fp8, alpha=alpha)
    in_maps = _prep_inputs(x_current, x_prev, alpha, connection_values,
                           connection_indices, fp8)
    res = run_bass_kernel_spmd(nc, in_maps, list(range(NCORES)))
    out = np.concatenate([res.results[i]["out"] for i in range(NCORES)], axis=0)
    return out.reshape(B, S, H).astype(np.float32, copy=False)


# revision 19
# speedup vs baseline: 1.1013x; 1.0015x over previous
"""Trainium2 kernel for nn_CrossLayerLateral.

out[b,s,i] = x_current[b,s,i] + alpha * sum_j x_prev[b,s,j] * W[i,j]
with W built from COO (duplicates summed).

Strategy (data-parallel over tokens, 8 cores):
  - Host: build WT[j,i] = W[i,j] via bincount scatter-add (cheap, O(nnz));
    cast WT and x_prev to a low-precision matmul dtype. The quantization
    error lands only on the alpha-scaled lateral term (~1.4e-4 of |out|),
    so even fp8 keeps the final output within ~1e-5 relative.
  - Flatten (B,S) -> 8192 tokens, 1024 tokens per core. Each core gets:
      xc  [1024, 2048] f32       x_current slice (token-major)
      wx  [16, 128, 3072] lp     per j-chunk: [scaled W^T row | x_prev^T],
                                 j on partitions, W^T replicated
      out [1024, 2048] f32
  - Device: psum = xpt.T @ wt accumulated over j-chunks in PSUM (fp32),
    DVE fused (psum * scale + xc), DMA out per 512-column block.
  - Token tiles are processed in pairs (8 psum groups = all 8 banks), so
    during the weight-streaming phase the PE gets 8 matmuls of work per
    arriving j-chunk.
  - FP8 path (default): e4m3 operands with perf_mode=DoubleRow (virtual
    128x256 array) halves the matmul stream; W^T is pre-scaled by 2**10
    into the e4m3 normal range and alpha/2**10 is folded into the DVE op.
"""
import numpy as np
import ml_dtypes

import concourse.bass as bass
import concourse.tile as tile
from concourse import bacc, mybir
from concourse.bass_utils import run_bass_kernel_spmd

H = 2048          # hidden
B, S = 4, 2048
TOK = B * S       # 8192 tokens
NCORES = 8
TPC = TOK // NCORES   # 1024 tokens per core
P = 128
JC = H // P       # 16 j-chunks (contraction)
ST = TPC // P     # 8 token tiles per core
NB = 512          # matmul free dim / psum bank
IB = H // NB      # 4 output column blocks
NBLK = ST // 2    # token-tile pairs
CW = H + TPC      # combined per-chunk row: [wt row | xpt row]
WARMUP_MM = 12    # HAM warmup matmuls while weights stream in

FP8 = True        # False -> bf16 matmul path
W_SCALE = 1024.0  # fp8: pre-scale for W^T into e4m3 normal range

_NC_CACHE = {}


def build_nc(fp8=FP8, alpha=1.0):
    lp_dt = mybir.dt.float8e4 if fp8 else mybir.dt.bfloat16
    perf_mode = mybir.MatmulPerfMode.DoubleRow if fp8 else None
    kstep = 2 if fp8 else 1   # j-chunks consumed per matmul

    nc = bacc.Bacc("TRN2", target_bir_lowering=False, debug=False,
                   enable_asserts=False, num_devices=NCORES)
    xc = nc.dram_tensor("xc", [TPC, H], mybir.dt.float32,
                        kind="ExternalInput").ap()
    wx = nc.dram_tensor("wx", [JC, P, CW], lp_dt,
                        kind="ExternalInput").ap()
    out = nc.dram_tensor("out", [TPC, H], mybir.dt.float32,
                         kind="ExternalOutput").ap()

    with tile.TileContext(nc) as tc:
        with (
            tc.tile_pool(name="weights", bufs=1) as wpool,
            tc.tile_pool(name="io", bufs=3) as io,
            tc.tile_pool(name="psum", bufs=2, space="PSUM") as psum,
        ):
            # PE warmup: garbage matmuls with no DMA deps keep the HAM
            # activity window busy while the first weight chunks stream in.
            wu = wpool.tile([P, P], mybir.dt.bfloat16, name="wu", tag="wu")
            nc.vector.memset(wu[:], 0.0)
            wu_ps = psum.tile([P, P], mybir.dt.float32, name="wups",
                              tag="ps0")
            for _ in range(WARMUP_MM):
                nc.tensor.matmul(wu_ps[:], lhsT=wu[:], rhs=wu[:],
                                 start=True, stop=True)

            # Resident weights+activations in one 3D tile (DoubleRow needs
            # [P, 2, n] APs spanning adjacent j-chunks), DMA'd per j-chunk
            # so matmuls start as chunks land. Alternate the two HWDGE
            # issue engines (sync / scalar). x_current tiles for the first
            # block are slotted into the weight stream so the first adds
            # aren't gated on the tail of the weight transfer.
            wx_sb = wpool.tile([P, JC, CW], lp_dt, name="wxs", tag="wxs")
            xc_pend = {}

            def load_xc(st, eng):
                t = io.tile([P, H], mybir.dt.float32, name=f"xc{st}",
                            tag=f"xc{st % 2}")
                eng.dma_start(t[:], xc[bass.ts(st, P), :])
                return t

            for jo in range(JC):
                eng = nc.sync if jo % 2 == 0 else nc.scalar
                eng.dma_start(wx_sb[:, jo, :], wx[jo])
            xc_pend[0] = load_xc(0, nc.sync)
            xc_pend[1] = load_xc(1, nc.scalar)

            for blk in range(NBLK):
                sts = (2 * blk, 2 * blk + 1)
                xc_ts = []
                for k, st in enumerate(sts):
                    if st in xc_pend:
                        xc_ts.append(xc_pend.pop(st))
                    else:
                        xc_ts.append(load_xc(st, nc.sync if k == 0
                                             else nc.scalar))
                out_ts = [io.tile([P, H], mybir.dt.float32, name=f"o{st}",
                                  tag=f"out{k}") for k, st in enumerate(sts)]
                # 8 psum groups per block over 4 tags x 2 slots = 8 banks
                ps = [psum.tile([P, NB], mybir.dt.float32,
                                name=f"ps{blk}_{g}", tag=f"ps{g % 4}")
                      for g in range(2 * IB)]

                def mm(g, jp):
                    k, ib = divmod(g, IB)
                    jsl = slice(jp * kstep, (jp + 1) * kstep)
                    lhsT = wx_sb[:, jsl, H + sts[k] * P:H + (sts[k] + 1) * P]
                    rhs = wx_sb[:, jsl, ib * NB:(ib + 1) * NB]
                    if not fp8:
                        lhsT = lhsT[:, 0]
                        rhs = rhs[:, 0]
                    nc.tensor.matmul(
                        ps[g][:], lhsT=lhsT, rhs=rhs,
                        start=(jp == 0), stop=(jp == JC // kstep - 1),
                        perf_mode=perf_mode,
                    )

                if blk == 0:
                    # Weight-streaming phase: consume j-chunks in arrival
                    # order, 8 matmuls of work per chunk.
                    for jp in range(JC // kstep):
                        for g in range(2 * IB):
                            mm(g, jp)
                else:
                    # Resident phase: finish groups one at a time so the
                    # adds/stores stagger across the whole block.
                    for g in range(2 * IB):
                        for jp in range(JC // kstep):
                            mm(g, jp)

                # Block A only: two-step epilogue — the scaled psum->sbuf
                # copy frees the bank without waiting for x_current (which
                # trails the weight stream in the DMA queue), so block B's
                # matmuls start immediately and HAM stays warm. Later
                # blocks use the fused (psum*scale + xc) op: their xc
                # tiles land well before their matmuls finish.
                if blk == 0:
                    for g in range(2 * IB):
                        k, ib = divmod(g, IB)
                        isl = bass.ts(ib, NB)
                        if fp8:
                            nc.vector.tensor_scalar_mul(
                                out_ts[k][:, isl], ps[g][:],
                                float(alpha) / W_SCALE)
                        else:
                            nc.vector.tensor_copy(out_ts[k][:, isl],
                                                  ps[g][:])
                    for g in range(2 * IB):
                        k, ib = divmod(g, IB)
                        isl = bass.ts(ib, NB)
                        nc.vector.tensor_add(out_ts[k][:, isl],
                                             out_ts[k][:, isl],
                                             xc_ts[k][:, isl])
                        eng = nc.sync if g % 2 == 0 else nc.scalar
                        eng.dma_start(out[bass.ts(sts[k], P), isl],
                                      out_ts[k][:, isl])
                else:
                    for g in range(2 * IB):
                        k, ib = divmod(g, IB)
                        isl = bass.ts(ib, NB)
                        if fp8:
                            nc.vector.scalar_tensor_tensor(
                                out_ts[k][:, isl], ps[g][:],
                                float(alpha) / W_SCALE, xc_ts[k][:, isl],
                                mybir.AluOpType.mult, mybir.AluOpType.add)
                        else:
                            nc.vector.tensor_add(out_ts[k][:, isl],
                                                 ps[g][:], xc_ts[k][:, isl])
                        eng = nc.sync if g % 2 == 0 else nc.scalar
                        eng.dma_start(out[bass.ts(sts[k], P), isl],
                                      out_ts[k][:, isl])

    nc.compile()
    return nc


def _get_nc(fp8=FP8, alpha=1.0):
    # alpha is baked into the fp8 epilogue immediate, so key the cache on it
    key = (fp8, float(alpha) if fp8 else None)
    if key not in _NC_CACHE:
        _NC_CACHE[key] = build_nc(fp8, alpha=alpha)
    return _NC_CACHE[key]


def _prep_inputs(x_current, x_prev, alpha, connection_values,
                 connection_indices, fp8=FP8):
    lp_np = mybir.dt.np(mybir.dt.float8e4) if fp8 else ml_dtypes.bfloat16
    # WT[j, i] = W[i, j];  W[r, c] += v  =>  WT[c, r] += v
    r = connection_indices[0].astype(np.int64)
    c = connection_indices[1].astype(np.int64)
    wt_flat = np.bincount(c * H + r, weights=connection_values.astype(np.float64),
                          minlength=H * H).astype(np.float32)
    if fp8:
        wt_lp = (np.float32(W_SCALE) * wt_flat).astype(lp_np).reshape(H, H)
    else:
        # fold alpha into the bf16 weights; DVE does a plain add
        wt_lp = (np.float32(alpha) * wt_flat).astype(lp_np).reshape(H, H)

    xp = np.asarray(x_prev, dtype=np.float32).reshape(TOK, H)
    xc = np.ascontiguousarray(np.asarray(x_current,
                                         dtype=np.float32).reshape(TOK, H))
    in_maps = []
    for core in range(NCORES):
        sl = slice(core * TPC, (core + 1) * TPC)
        wx_core = np.empty((H, CW), dtype=lp_np)
        wx_core[:, :H] = wt_lp
        wx_core[:, H:] = xp[sl].T.astype(lp_np)
        in_maps.append({
            "xc": np.ascontiguousarray(xc[sl]),
            "wx": wx_core.reshape(JC, P, CW),
        })
    return in_maps


def _have_devices():
    # The bass run needs the 8 NeuronCores on jax's default platform. If
    # the calling process pinned jax to cpu before we got here, try to
    # re-resolve; report False if the cores stay invisible.
    import jax
    if len(jax.devices()) >= NCORES:
        return True
    try:
        from jax._src import xla_bridge as xb
        jax.config.update("jax_platforms", "axon,cpu")
        xb._clear_backends()
    except Exception:
        pass
    return len(jax.devices()) >= NCORES


def _kernel_subprocess(**inputs):
    # Fallback when this process's jax was already initialized without the
    # NeuronCores: run in a fresh interpreter (sitecustomize re-registers
    # the device platform there).
    import os
    import subprocess
    import sys
    import tempfile

    here = os.path.dirname(os.path.abspath(__file__))
    with tempfile.TemporaryDirectory() as td:
        np.savez(os.path.join(td, "in.npz"), **inputs)
        script = (
            "import sys, numpy as np; sys.path.insert(0, %r); "
            "import kernel as K; d = np.load(%r); "
            "out = K.kernel(**{k: d[k] for k in d.files}); "
            "np.save(%r, out)"
            % (here, os.path.join(td, "in.npz"), os.path.join(td, "out.npy"))
        )
        env = dict(os.environ)
        env.pop("JAX_PLATFORMS", None)
        env["BASS_KERNEL_SUBPROC"] = "1"
        subprocess.run([sys.executable, "-c", script], check=True, env=env)
        return np.load(os.path.join(td, "out.npy"))


def kernel(x_current, x_prev, alpha, connection_values, connection_indices):
    import os
    fp8 = FP8
    inputs = dict(x_current=x_current, x_prev=x_prev, alpha=alpha,
                  connection_values=connection_values,
                  connection_indices=connection_indices)
    if not _have_devices():
        if os.environ.get("BASS_KERNEL_SUBPROC"):
            raise RuntimeError(f"need {NCORES} NeuronCores on the default "
                               "jax platform; none visible")
        return _kernel_subprocess(**inputs)
    nc = _get_nc(fp8, alpha=alpha)
    in_maps = _prep_inputs(fp8=fp8, **inputs)
    res = run_bass_kernel_spmd(nc, in_maps, list(range(NCORES)))
    out = np.concatenate([res.results[i]["out"] for i in range(NCORES)], axis=0)
    return out.reshape(B, S, H).astype(np.float32, copy=False)
